# revision 5
# baseline (speedup 1.0000x reference)
"""AuxSeLoss v4: the v1 dataflow (ACT softplus + DVE dots/t-reduce in
parallel; PE only for tiny stat folds) with overhead trims:
  - F=5376 chunks: 26 ACTIVATE instead of 34, 13 accumulator reads
    instead of 17 (the reads cost 279ns each on the critical ACT queue).
  - dot outputs overwrite the spent e tiles, freeing SBUF for the big
    chunks (no separate garbage tile).
  - per-chunk stats fold into PSUM via a ones-matmul with start/stop, so
    the tail is one copy + DMA (no V collapse).
  - chunk-0 input DMA is triggered before the out2 warmup DMA, so the
    bulk stream starts ~1us earlier; the warmup still forces the single
    exp/ln ACT table load to happen early.
Lesson from v2/v3: adding GpSimd or PE bulk work oversubscribes the SBUF
ports and slows DVE/ACT/DMA below their solo rates -- so everything heavy
stays exactly where v1 had it.
"""

import numpy as np

N_CLASSES = 21
B, C, H, W = 16, N_CLASSES, 256, 256
N_CORES = 8
B_LOCAL = B // N_CORES
ELEMS_PER_SAMPLE = C * H * W  # 1376256
P = 128
FREE_PER_SAMPLE = ELEMS_PER_SAMPLE // P  # 10752
ROWS = B_LOCAL * P
AUX_WEIGHT = 0.4
SE_WEIGHT = 0.2
N_TOTAL = B * C * H * W
N_SE = B * C

CHUNK_SCHEDULE = [
    [672, 5376, 4704],  # sample 0: small first chunk -> fast ACT start
    [5376, 5040, 336],  # sample 1: tiny last chunk -> short tail
]
assert all(sum(cs) == FREE_PER_SAMPLE for cs in CHUNK_SCHEDULE)
FMAX = 5376
FALLOC = 5448  # pad tiles +288B so DMA/engine SBUF bank phases differ
NSTAT = 8  # Vc cols per sample half: 0=sp0 1=sp1 2=d0 3=d1 4=tsum 7=sp2

_CACHE: dict = {}


def _build():
    from contextlib import ExitStack

    import concourse.bacc as bacc
    import concourse.mybir as mybir
    from concourse.tile import TileContext

    f32 = mybir.dt.float32
    AFT = mybir.ActivationFunctionType
    ALU = mybir.AluOpType

    import concourse.hw_specs as hw_specs

    tables = hw_specs.get_activation_tables("gen3")
    combined = "natural_log_exp_and_others"
    if combined in tables and {AFT.Exp, AFT.Ln} <= tables[combined]:
        for name, funcs in tables.items():
            if name != combined:
                funcs.discard(AFT.Exp)
                funcs.discard(AFT.Ln)

    nc = bacc.Bacc("TRN2", target_bir_lowering=False)
    x0 = nc.dram_tensor("out0", [ROWS, FREE_PER_SAMPLE], f32, kind="ExternalInput")
    x1 = nc.dram_tensor("out1", [ROWS, FREE_PER_SAMPLE], f32, kind="ExternalInput")
    tg = nc.dram_tensor("targets", [ROWS, FREE_PER_SAMPLE], f32, kind="ExternalInput")
    o2 = nc.dram_tensor("out2", [1, B_LOCAL * C], f32, kind="ExternalInput")
    res = nc.dram_tensor("stats", [1, 16], f32, kind="ExternalOutput")

    with ExitStack() as ctx, TileContext(nc) as tc:
        with (
            tc.tile_pool(name="tp", bufs=2) as tp,
            tc.tile_pool(name="x0p", bufs=2) as x0p,
            tc.tile_pool(name="x1p", bufs=2) as x1p,
            tc.tile_pool(name="ep", bufs=2) as ep,
            tc.tile_pool(name="gdp", bufs=1) as gdp,
            tc.tile_pool(name="vcp", bufs=2) as vcp,
            tc.tile_pool(name="accp", bufs=1) as accp,
            tc.tile_pool(name="psp", bufs=1, space="PSUM") as psp,
        ):
            ones_t = accp.tile([P, 1], f32)
            nc.vector.memset(ones_t[:], 1.0)
            Us = accp.tile([1, 16], f32)
            o2_t = accp.tile([1, B_LOCAL * C], f32)
            e_o2 = accp.tile([1, B_LOCAL * C], f32)
            g_o2 = accp.tile([1, B_LOCAL * C], f32)
            U = psp.tile([1, 16], f32)

            chunks = []
            for s in range(B_LOCAL):
                c0 = 0
                for cols in CHUNK_SCHEDULE[s]:
                    chunks.append((s, c0, cols))
                    c0 += cols
            n_chunks = len(chunks)

            # x0 tiles are allocated/DMA'd one chunk ahead so the ACT
            # engine (whose first op per chunk reads x0) never waits:
            # issue order: x0(c0), then per chunk c: t(c), x1(c), x0(c+1).
            # Depth-1 keeps the x0(c+1) trigger's buffer-wait (on compute of
            # c-1) already satisfied when the sync engine reaches it.
            x0_tiles = []

            def x0_tile(ci):
                s, c0, cols = chunks[ci]
                r0, r1 = s * P, (s + 1) * P
                xt = x0p.tile([P, FALLOC], f32, name=f"x0_{ci}", tag="x0")
                nc.sync.dma_start(xt[:, 0:cols], x0[r0:r1, c0 : c0 + cols])
                x0_tiles.append(xt)
                return xt

            x0_tile(0)
            first = True
            for ci, (s, c0, cols) in enumerate(chunks):
                r0, r1 = s * P, (s + 1) * P
                c1 = c0 + cols
                t_t = tp.tile([P, FALLOC], f32, name=f"t_{ci}", tag="t")
                x0_t = x0_tiles[ci]
                x1_t = x1p.tile([P, FALLOC], f32, name=f"x1_{ci}", tag="x1")
                nc.sync.dma_start(t_t[:, 0:cols], tg[r0:r1, c0:c1])
                nc.sync.dma_start(x1_t[:, 0:cols], x1[r0:r1, c0:c1])
                if ci + 1 < len(chunks):
                    x0_tile(ci + 1)

                Vc = vcp.tile([P, 16], f32, name=f"vc_{ci}", tag="vc")
                nc.gpsimd.memset(Vc[:], 0.0)
                sc = s * NSTAT

                if first:
                    # out2 DMA + softplus warmup: forces the act table load
                    # early; issued after chunk-0's bulk DMA triggers.
                    nc.sync.dma_start(o2_t[:], o2[0:1, :])
                    nc.scalar.activation(e_o2[:], o2_t[:], AFT.Exp)
                    nc.scalar.activation(
                        g_o2[:], e_o2[:], AFT.Ln, bias=1.0,
                        accum_out=Vc[0:1, sc + 7 : sc + 8],
                    )
                    first = False

                e0 = ep.tile([P, FALLOC], f32, name=f"e0_{ci}", tag="e")
                e1 = ep.tile([P, FALLOC], f32, name=f"e1_{ci}", tag="e")
                # ACT: softplus via exp then ln(1+.) in place, sum fused
                # into the activation accumulator.
                nc.scalar.activation(e0[:, 0:cols], x0_t[:, 0:cols], AFT.Exp)
                nc.scalar.activation(
                    e0[:, 0:cols], e0[:, 0:cols], AFT.Ln, bias=1.0,
                    accum_out=Vc[:, sc + 0 : sc + 1],
                )
                nc.scalar.activation(e1[:, 0:cols], x1_t[:, 0:cols], AFT.Exp)
                nc.scalar.activation(
                    e1[:, 0:cols], e1[:, 0:cols], AFT.Ln, bias=1.0,
                    accum_out=Vc[:, sc + 1 : sc + 2],
                )

                # DVE: x.t dots into an independent scratch tile so they
                # run in parallel with ACT, plus the exact per-sample t sum.
                gd = gdp.tile([P, FALLOC], f32, name=f"gd_{ci}", tag="gd")
                nc.vector.scalar_tensor_tensor(
                    out=gd[:, 0:cols], in0=x0_t[:, 0:cols], scalar=1.0,
                    in1=t_t[:, 0:cols], op0=ALU.mult, op1=ALU.mult,
                    accum_out=Vc[:, sc + 2 : sc + 3],
                )
                nc.vector.scalar_tensor_tensor(
                    out=gd[:, 0:cols], in0=x1_t[:, 0:cols], scalar=1.0,
                    in1=t_t[:, 0:cols], op0=ALU.mult, op1=ALU.mult,
                    accum_out=Vc[:, sc + 3 : sc + 4],
                )
                nc.vector.tensor_reduce(
                    out=Vc[:, sc + 4 : sc + 5], in_=t_t[:, 0:cols],
                    axis=mybir.AxisListType.X, op=ALU.add,
                )

                # Fold this chunk's stats into PSUM (ones-matmul is exact
                # in fp32r for these integer-ish sums).
                nc.tensor.matmul(
                    U[:], ones_t[:], Vc[:],
                    start=(ci == 0), stop=(ci == n_chunks - 1),
                )

            nc.vector.tensor_copy(Us[:], U[:])
            nc.sync.dma_start(res[0:1, :], Us[:])

    nc.finalize()
    return nc


def _get_nc():
    if "nc" not in _CACHE:
        _CACHE["nc"] = _build()
    return _CACHE["nc"]


def _run(in_maps, trace=False):
    from concourse.bass_utils import run_bass_kernel_spmd

    return run_bass_kernel_spmd(
        _get_nc(), in_maps, core_ids=list(range(N_CORES)), trace=trace
    )


def make_in_maps(out0, out1, out2, targets):
    in_maps = []
    for c in range(N_CORES):
        sl = slice(c * B_LOCAL, (c + 1) * B_LOCAL)
        in_maps.append(
            {
                "out0": np.ascontiguousarray(out0[sl]).reshape(ROWS, FREE_PER_SAMPLE),
                "out1": np.ascontiguousarray(out1[sl]).reshape(ROWS, FREE_PER_SAMPLE),
                "targets": np.ascontiguousarray(targets[sl]).reshape(
                    ROWS, FREE_PER_SAMPLE
                ),
                "out2": np.ascontiguousarray(out2[sl]).reshape(1, B_LOCAL * C),
            }
        )
    return in_maps


def combine_partials(stats, out2):
    """Host-side O(1) combine. stats: [N_CORES, 16] per-core sums."""
    total_main = 0.0
    total_se = 0.0
    for c in range(len(stats)):
        v = [float(x) for x in stats[c]]
        total_se += v[7]  # sp2
        for s in range(B_LOCAL):
            o = s * NSTAT
            total_main += (v[o + 0] - v[o + 2]) + AUX_WEIGHT * (v[o + 1] - v[o + 3])
            t_sum = v[o + 4]
            b_global = c * B_LOCAL + s
            if t_sum < ELEMS_PER_SAMPLE - 0.5:  # class-bin 0 present
                total_se -= float(out2[b_global, 0])
            if t_sum > 0.5:  # class-bin 1 present
                total_se -= float(out2[b_global, 1])
    return total_main / N_TOTAL + SE_WEIGHT * total_se / N_SE


def kernel(out0, out1, out2, targets):
    out0 = np.asarray(out0, dtype=np.float32)
    out1 = np.asarray(out1, dtype=np.float32)
    out2 = np.asarray(out2, dtype=np.float32)
    targets = np.asarray(targets, dtype=np.float32)
    br = _run(make_in_maps(out0, out1, out2, targets))
    stats = [r["stats"][0] for r in br.results]
    return np.asarray(combine_partials(stats, out2), dtype=np.float32)
